# revision 57
# baseline (speedup 1.0000x reference)
"""Multi-head self-attention TRN2 Bass kernel.

Problem: x[4,2048,512], 8 heads of d=64, scale 1/sqrt(512) (full feature dim).

Sharding: 8 cores = (batch b in 0..3) x (head-group hg in 0..1). Each core
handles one batch element and 4 heads (256 of the 512 features), computing a
partial output projection z_partial = attn_heads @ Wo[hg rows].  The host
sums the two partials per batch and adds bo.

Per-core dataflow (ACT exp is the bottleneck; PE work minimized):
  prologue: xt [512,2048] (host pre-transposed, bf16) and packed bf16
            W_{k,q,v} streamed in; QT/KT = W^T x^T per head-pair [128, n]
            (+bias per partition, accumulated f32);
            V per j-tile [128, 260] bf16 with a ones column per head.
  rounds (icb 0..1 i-chunks of 1024) x (head pair p 0..1), jt 0..15:
    S^T_h [128j, 1024i] = K_h Q_h^T  (two heads row-packed via tile_position)
    P^T_h = exp(S^T_h / sqrt(512))   (ScalarE -> bf16 SBUF)
    O[i_sub, 65] += P^T_slice.T V_ext  (lhsT = P^T 128x128 slice, rhs = V_ext
       [128, 65]; col 64 accumulates the softmax row-sum -> per-partition!)
  normalize: rcp = 1/O[:,64] (DVE [128,1]); O_norm = O[:, :64] * rcp
    (tensor_scalar, per-partition broadcast) -> staged [128 i, 128 d] bf16.
  O^T via identity-matmul transpose (PE) + DVE copy -> ot[kt][128 d, n] bf16;
  z^T[f,i] = Wo^T O^T (bf16) + bias, DMA out per 512-chunk.

Scheduling: ACT must never stall. S(jt+1) is emitted before PV(jt); all other
work (projections, transposes of the previous round, z chunks) lives in a
background FIFO popped once per jt AFTER S(jt+1,h1), so a stalled item can
only delay work that has >1 jt of slack. Background items chain through the
single spare PSUM bank (pz); items are ordered so each completes before its
consumer's deadline.

Output zt [512, 2048] = z^T; host transposes back, sums partials, adds bo.
"""

import sys
import os

sys.path.insert(0, "/opt/trn_rl_repo")

import numpy as np

B, N, F = 4, 2048, 512
H, D = 8, 64
P = 128
DH = 256   # features per core (4 heads)
NPAIR = 2  # head pairs per core
KT = F // P          # 4 k-tiles over input features
ICB = 1024           # i-chunk per round
NICB = N // ICB      # 2
NT = N // P          # 16 j-tiles
NSUB = ICB // P      # 8 i-subtiles per chunk
SCALE = 1.0 / float(np.float32(F) ** 0.5)

_cache = {}


def _bf_np():
    import ml_dtypes

    return np.dtype(ml_dtypes.bfloat16)


def build():
    """Build + bass-compile the per-core program."""
    import concourse.tile as tile
    from concourse import bacc, mybir
    from contextlib import ExitStack

    f32 = mybir.dt.float32
    f32r = mybir.dt.float32r
    bf = mybir.dt.bfloat16
    AF = mybir.ActivationFunctionType

    n = N
    nc = bacc.Bacc("TRN2", target_bir_lowering=False, debug=False)

    xt_d = nc.dram_tensor("xtb", [F, n], bf, kind="ExternalInput").ap()
    wkqv_d = nc.dram_tensor("wkqv", [F, 3 * DH], bf, kind="ExternalInput").ap()
    wo_d = nc.dram_tensor("wob", [DH, F], bf, kind="ExternalInput").ap()
    # packed biases: cols 0-1 bk, 2-3 bq, 4-7 bv@Wo (one DMA, not three)
    bias_d = nc.dram_tensor("bias8", [P, 8], f32, kind="ExternalInput").ap()
    id_d = nc.dram_tensor("ident", [P, P], bf, kind="ExternalInput").ap()
    zt_d = nc.dram_tensor("zt", [F, n], f32, kind="ExternalOutput").ap()

    with tile.TileContext(nc) as tc, ExitStack() as ctx:
        const = ctx.enter_context(tc.tile_pool(name="const", bufs=1))
        pt_pool = ctx.enter_context(tc.tile_pool(name="pt", bufs=6))
        rc_pool = ctx.enter_context(tc.tile_pool(name="rc", bufs=8))
        on_pool = ctx.enter_context(tc.tile_pool(name="on", bufs=16))
        zs_pool = ctx.enter_context(tc.tile_pool(name="zs", bufs=4))
        ps_s = ctx.enter_context(tc.tile_pool(name="ps_s", bufs=2, space="PSUM"))
        po_pool = ctx.enter_context(tc.tile_pool(name="po", bufs=3, space="PSUM"))
        pz_pool = ctx.enter_context(tc.tile_pool(name="pz", bufs=1, space="PSUM"))

        # ---- DMA loads: bf16 x and packed bf16 W_{k,q,v} minimize both the
        # transfer bytes and the HWDGE-serialized DMA count on the critical
        # path to the first exp ----------------------------------------------
        xt = [const.tile([P, n], bf, tag=f"xt{k}", name=f"xt{k}") for k in range(KT)]
        wkqv = [const.tile([P, 3 * DH], bf, tag=f"wkqv{k}", name=f"wkqv{k}")
                for k in range(KT)]
        wk = [wkqv[k][:, 0:DH] for k in range(KT)]
        wq = [wkqv[k][:, DH:2 * DH] for k in range(KT)]
        wv = [wkqv[k][:, 2 * DH:3 * DH] for k in range(KT)]
        bias_sb = const.tile([P, 8], f32, tag="bias8", name="bias_sb")
        bk_sb = bias_sb[:, 0:NPAIR]
        bq_sb = bias_sb[:, NPAIR:2 * NPAIR]
        zb_sb = bias_sb[:, 4:8]
        # warm tiles first: a dummy matmul starts the PE p-state ramp at t~1us
        # so the prologue projections run at full clock, and the exp warms the
        # ScalarE table while DMAs stream in
        warm = const.tile([1, 1], f32, tag="warm", name="warm")
        nc.vector.memset(warm[:], 0.0)
        warmmm = pz_pool.tile([P, 512], f32, tag="pz", name="warmmm")
        nc.tensor.matmul(warmmm[0:1, 0:1], warm[:], warm[:],
                         start=True, stop=True)
        nc.scalar.activation(warm[:], warm[:], AF.Exp)

        nc.sync.dma_start(xt[0][:, 0:ICB], xt_d[0:P, 0:ICB])
        nc.sync.dma_start(wkqv[0][:], wkqv_d[0:P, :])
        nc.sync.dma_start(bias_sb[:], bias_d[:])
        for k in range(1, KT):
            nc.sync.dma_start(xt[k][:, 0:ICB], xt_d[k * P:(k + 1) * P, 0:ICB])
            nc.sync.dma_start(wkqv[k][:], wkqv_d[k * P:(k + 1) * P, :])
        for k in range(KT):
            nc.sync.dma_start(xt[k][:, 1024:n], xt_d[k * P:(k + 1) * P, 1024:n])
        wo = [const.tile([P, F], bf, tag=f"wo{k}", name=f"wo{k}") for k in range(DH // P)]
        for k in range(DH // P):
            nc.sync.dma_start(wo[k][:], wo_d[k * P:(k + 1) * P, :])
        ident = const.tile([P, P], bf, tag="ident", name="ident")
        nc.sync.dma_start(ident[:], id_d[:])

        # persistent activations
        qt = [const.tile([P, n], f32r, tag=f"qt{p}", name=f"qt{p}") for p in range(NPAIR)]
        kt_sb = [const.tile([P, n], f32r, tag=f"kt{p}", name=f"ktsb{p}") for p in range(NPAIR)]
        # V per j-tile: [128, 260] bf16, head hl at cols [65*hl, 65*hl+64),
        # ones at col 65*hl+64 (accumulates softmax row-sums in PV).
        v_sb = [const.tile([P, 4 * (D + 1)], bf, tag=f"v{j}", name=f"v{j}")
                for j in range(NT)]
        for j in range(NT):
            nc.gpsimd.memset(v_sb[j][:], 1.0)
        # O^T staging for the z projection: [128 d, n] bf16 per k-tile (pair)
        ot = [const.tile([P, n], bf, tag=f"ot{p}", name=f"ot{p}") for p in range(NPAIR)]

        def v4(ap):
            return ap.rearrange("p (h c) -> p h c", h=4)

        # ---- projections -----------------------------------------------------
        def proj_qk_half(p, w_t, b_sb, dst, ib, half, pool, act_bias=False):
            """One 512-wide half of a Q/K projection chunk."""
            ps = pool.tile([P, 512], f32, tag="pz", name="pjh") if pool is pz_pool \
                else pool.tile([P, ICB], f32, tag="st", name="pjs")
            psl = ps[:, 0:512]
            isl = slice(ib * ICB + half * 512, ib * ICB + (half + 1) * 512)
            for k in range(KT):
                nc.tensor.matmul(
                    psl,
                    w_t[k][:, p * P:(p + 1) * P],
                    xt[k][:, isl],
                    start=(k == 0),
                    stop=(k == KT - 1),
                )
            if act_bias:
                # prologue: ACT is idle and this sits on the first-exp path
                nc.scalar.activation(dst[p][:, isl], psl, AF.Identity,
                                     bias=b_sb[:, p:p + 1])
            else:
                nc.vector.tensor_scalar_add(dst[p][:, isl], psl, b_sb[:, p:p + 1])

        def proj_v_pair(j0, pool):
            """V for j-tiles j0, j0+1 sharing one pz bank (two 256-col slices)."""
            ps = pool.tile([P, 512], f32, tag="pz", name="pvh") if pool is pz_pool \
                else pool.tile([P, ICB], f32, tag="st", name="pvs")
            for m, j in enumerate((j0, j0 + 1)):
                psl = ps[:, m * DH:(m + 1) * DH]
                for k in range(KT):
                    # one start=True per pz bank; the second slice's first
                    # matmul writes pending-zero bytes (zeroed on write)
                    nc.tensor.matmul(
                        psl,
                        xt[k][:, j * P:(j + 1) * P],
                        wv[k][:],
                        start=(k == 0 and m == 0),
                        stop=(k == KT - 1 and m == 1),
                        skip_group_check=True,
                    )
            for m, j in enumerate((j0, j0 + 1)):
                psl = ps[:, m * DH:(m + 1) * DH]
                nc.vector.tensor_copy(v4(v_sb[j][:])[:, :, 0:D], v4(psl))

        def z_chunk(icb, ft, ch, pool):
            """z^T[ft*128:(ft+1)*128, 512-chunk ch of icb]."""
            if pool is pz_pool:
                zp = pool.tile([P, 512], f32, tag="pz", name="zp")
                zpl = zp[:, 0:512]
            else:
                zp = pool.tile([P, ICB], f32, tag="st", name="zps")
                zpl = zp[:, 0:512]
            isl = slice(icb * ICB + ch * 512, icb * ICB + (ch + 1) * 512)
            for k in range(DH // P):
                nc.tensor.matmul(
                    zpl,
                    wo[k][:, ft * P:(ft + 1) * P],
                    ot[k][:, isl],
                    start=(k == 0),
                    stop=(k == DH // P - 1),
                )
            zsb = zs_pool.tile([P, 512], f32, tag="zt", name="zsb")
            nc.vector.tensor_scalar_add(zsb[:], zpl, zb_sb[:, ft:ft + 1])
            nc.sync.dma_start(zt_d[ft * P:(ft + 1) * P, isl], zsb[:])

        def transp_group(p, icb, onst, grp):
            """Transpose 4 O_norm [128,64] blocks -> ot via one pz bank."""
            tp = pz_pool.tile([P, 512], f32, tag="pz", name="tp")
            for i, (h, sub) in enumerate(grp):
                nc.tensor.matmul(
                    tp[0:D, i * P:(i + 1) * P],
                    onst[sub][:, h * D:(h + 1) * D],
                    ident[:],
                    start=(i == 0),
                    stop=(i == 3),
                    skip_group_check=True,
                )
            for i, (h, sub) in enumerate(grp):
                dst = ot[p][h * D:(h + 1) * D,
                            icb * ICB + sub * P: icb * ICB + (sub + 1) * P]
                nc.vector.tensor_copy(dst, tp[0:D, i * P:(i + 1) * P])

        # ---- attention rounds ------------------------------------------------
        def ob_slice(ob, h, sub):
            """PSUM accumulator slice [128, 65] for (head h, i-subtile sub)."""
            if sub < 7:
                t = ob[h]
                c0 = sub * (D + 1)
            else:
                t = ob[2]
                c0 = h * (D + 1)
            return t[:, c0:c0 + D + 1]

        def emit_S(p, icb, jt, h):
            sps = ps_s.tile([P, ICB], f32, tag="st", name="sps")
            hp = slice(D * h, D * (h + 1))
            for i5 in range(2):
                isl = slice(icb * ICB + i5 * 512, icb * ICB + (i5 + 1) * 512)
                nc.tensor.matmul(
                    sps[:, i5 * 512:(i5 + 1) * 512],
                    kt_sb[p][hp, jt * P:(jt + 1) * P],
                    qt[p][hp, isl],
                    start=True,
                    stop=True,
                    tile_position=(D * h, 0),
                )
            return sps

        # prologue: K/Q first chunks + the first S tile + the FIRST EXPS,
        # emitted as early as possible in the stream (the exps must precede
        # all other PE work in emission order or they inherit its position).
        # K-h1 goes through pz so the ps_s slots stay clear for the S tiles.
        proj_qk_half(0, wq, bq_sb, qt, 0, 0, ps_s, act_bias=True)  # slot A
        proj_qk_half(0, wk, bk_sb, kt_sb, 0, 0, ps_s)   # slot B
        sps00 = ps_s.tile([P, ICB], f32, tag="st", name="sps00")  # slot A
        nc.tensor.matmul(
            sps00[:, 0:512], kt_sb[0][0:D, 0:P], qt[0][0:D, 0:512],
            start=True, stop=True, tile_position=(0, 0),
        )
        ptile00 = pt_pool.tile([P, ICB], bf, tag="pt", name="pt00")
        nc.scalar.activation(ptile00[:, 0:512], sps00[:, 0:512],
                             AF.Exp, scale=SCALE)
        proj_qk_half(0, wq, bq_sb, qt, 0, 1, ps_s, act_bias=True)  # slot B
        nc.tensor.matmul(
            sps00[:, 512:1024], kt_sb[0][0:D, 0:P], qt[0][0:D, 512:1024],
            start=True, stop=True, tile_position=(0, 0),
        )
        proj_qk_half(0, wk, bk_sb, kt_sb, 0, 1, pz_pool)
        nc.scalar.activation(ptile00[:, 512:1024], sps00[:, 512:1024],
                             AF.Exp, scale=SCALE)
        sps01 = emit_S(0, 0, 0, 1)
        ptile01 = pt_pool.tile([P, ICB], bf, tag="pt", name="pt01")
        nc.scalar.activation(ptile01[:], sps01[:], AF.Exp, scale=SCALE)

        rounds = [(0, 0), (0, 1), (1, 0), (1, 1)]
        bg = []          # background FIFO: fns emitting pz-chained work
        onst_by_round = {}

        for r, (icb, p) in enumerate(rounds):
            # load this round's background work (deadline-ordered)
            if r == 0:
                bg += [
                    lambda: proj_v_pair(0, pz_pool),
                    lambda: proj_v_pair(2, pz_pool),
                    lambda: proj_v_pair(4, pz_pool),
                    lambda: proj_v_pair(6, pz_pool),
                    lambda: proj_v_pair(8, pz_pool),
                    lambda: proj_qk_half(0, wk, bk_sb, kt_sb, 1, 0, pz_pool),
                    lambda: proj_qk_half(0, wk, bk_sb, kt_sb, 1, 1, pz_pool),
                    lambda: proj_v_pair(10, pz_pool),
                    lambda: proj_v_pair(12, pz_pool),
                    lambda: proj_qk_half(1, wk, bk_sb, kt_sb, 0, 0, pz_pool),
                    lambda: proj_qk_half(1, wq, bq_sb, qt, 0, 0, pz_pool),
                    lambda: proj_v_pair(14, pz_pool),
                    lambda: proj_qk_half(1, wk, bk_sb, kt_sb, 0, 1, pz_pool),
                    lambda: proj_qk_half(1, wq, bq_sb, qt, 0, 1, pz_pool),
                ]
            elif r == 1:
                po_, oo_ = onst_by_round[0]
                items = [(h, s) for h in range(2) for s in range(NSUB)]
                bg += [
                    lambda: proj_qk_half(1, wk, bk_sb, kt_sb, 1, 0, pz_pool),
                    lambda: proj_qk_half(1, wk, bk_sb, kt_sb, 1, 1, pz_pool),
                ]
                bg += [lambda g=g, po2=po_, oo2=oo_: transp_group(
                        po2[0], po2[1], oo2, items[g * 4:(g + 1) * 4])
                       for g in range(4)]
                bg += [
                    lambda: proj_qk_half(0, wq, bq_sb, qt, 1, 0, pz_pool),
                    lambda: proj_qk_half(0, wq, bq_sb, qt, 1, 1, pz_pool),
                    lambda: proj_qk_half(1, wq, bq_sb, qt, 1, 0, pz_pool),
                    lambda: proj_qk_half(1, wq, bq_sb, qt, 1, 1, pz_pool),
                ]
            elif r == 2:
                po_, oo_ = onst_by_round[1]
                items = [(h, s) for h in range(2) for s in range(NSUB)]
                bg += [lambda g=g, po2=po_, oo2=oo_: transp_group(
                        po2[0], po2[1], oo2, items[g * 4:(g + 1) * 4])
                       for g in range(4)]
                bg += [lambda ft=ft, ch=ch: z_chunk(0, ft, ch, pz_pool)
                       for ft in range(F // P) for ch in range(2)]
            elif r == 3:
                po_, oo_ = onst_by_round[2]
                items = [(h, s) for h in range(2) for s in range(NSUB)]
                bg += [lambda g=g, po2=po_, oo2=oo_: transp_group(
                        po2[0], po2[1], oo2, items[g * 4:(g + 1) * 4])
                       for g in range(4)]

            ob = [po_pool.tile([P, 512], f32, tag="po", name=f"ob{i}")
                  for i in range(3)]
            if r == 0:
                sps_tiles = {0: sps00, 1: sps01}
            else:
                sps_tiles = {0: emit_S(p, icb, 0, 0), 1: emit_S(p, icb, 0, 1)}
            for jt in range(NT):
                nxt = {}
                if jt + 1 < NT:
                    nxt[0] = emit_S(p, icb, jt + 1, 0)
                    # slot 1: only pop when backlog exceeds remaining jts
                    if bg and len(bg) > (NT - jt):
                        bg.pop(0)()
                    nxt[1] = emit_S(p, icb, jt + 1, 1)
                if bg:
                    bg.pop(0)()
                for h in range(2):
                    hl = 2 * p + h
                    if r == 0 and jt == 0:
                        # exps pre-emitted in the prologue
                        ptile = (ptile00, ptile01)[h]
                    else:
                        ptile = pt_pool.tile([P, ICB], bf, tag="pt", name="pt")
                        nc.scalar.activation(ptile[:], sps_tiles[h][:], AF.Exp, scale=SCALE)
                    for sub in range(NSUB):
                        # start=True zeroes the whole 2KB psum bank row, so
                        # only the first matmul into each ob bank may set it
                        first_in_bank = (sub == 0) or (sub == 7 and h == 0)
                        nc.tensor.matmul(
                            ob_slice(ob, h, sub),
                            ptile[:, sub * P:(sub + 1) * P],
                            v_sb[jt][:, hl * (D + 1):(hl + 1) * (D + 1)],
                            start=(jt == 0 and first_in_bank),
                            stop=(jt == NT - 1),
                            skip_group_check=True,
                        )
                sps_tiles = nxt

            # ---- normalize (frees the ob banks for the next round) ----------
            onst = [on_pool.tile([P, P], bf, tag="on", name=f"on{s}")
                    for s in range(NSUB)]
            for h in range(2):
                # batched reciprocals: subs 0-6 rowsums (col 64 of each
                # 65-col slice in bank ob[h]) in one strided op, sub 7 single
                rc8 = rc_pool.tile([P, 8], f32, tag="rc8", name="rc8")
                in7 = ob[h][:, 0:7 * (D + 1)].rearrange(
                    "p (s c) -> p s c", c=D + 1)[:, :, D:D + 1]
                out7 = rc8[:, 0:7].rearrange("p (s c) -> p s c", c=1)
                nc.vector.reciprocal(out7, in7)
                nc.vector.reciprocal(
                    rc8[:, 7:8], ob[2][:, h * (D + 1) + D: h * (D + 1) + D + 1])
                for sub in range(NSUB):
                    sl = ob_slice(ob, h, sub)
                    rc = rc8[:, sub:sub + 1]
                    dst = onst[sub][:, h * D:(h + 1) * D]
                    if r == 3 and sub % 2 == h:
                        # tail: ACT is idle -> per-partition scale via Copy
                        nc.scalar.activation(dst, sl[:, 0:D], AF.Copy, scale=rc)
                    else:
                        nc.vector.tensor_scalar_mul(dst, sl[:, 0:D], rc)
            onst_by_round[r] = ((p, icb), onst)

        # ---- tail: last round's transposes via the freed wide ps_s tiles,
        # then icb=1 z chunks rotating through pz + both ps_s bufs ------------
        _, onst3 = onst_by_round[3]

        def tail_transp(half):
            """Transpose subs [4*half, 4*half+4) x both heads -> ot."""
            tp = ps_s.tile([P, ICB], f32, tag="st", name="tptail")
            grp8 = [(h, s) for h in range(2)
                    for s in range(half * 4, half * 4 + 4)]
            for i, (h, sub) in enumerate(grp8):
                nc.tensor.matmul(
                    tp[0:D, i * P:(i + 1) * P],
                    onst3[sub][:, h * D:(h + 1) * D],
                    ident[:],
                    start=(i % 4 == 0),
                    stop=(i % 4 == 3),
                    skip_group_check=True,
                )
            for i, (h, sub) in enumerate(grp8):
                dst = ot[1][h * D:(h + 1) * D,
                            ICB + sub * P: ICB + (sub + 1) * P]
                if i % 2 == 0:
                    nc.scalar.activation(dst, tp[0:D, i * P:(i + 1) * P], AF.Copy)
                else:
                    nc.vector.tensor_copy(dst, tp[0:D, i * P:(i + 1) * P])

        def tail_z(ft, ch, use_act, pool=None):
            """One [128,512] z chunk of icb=1 with ACT- or DVE-side bias."""
            if pool is pz_pool:
                zp = pz_pool.tile([P, 512], f32, tag="pz", name="zptail")
                zpl = zp[:, 0:512]
            else:
                zp = ps_s.tile([P, ICB], f32, tag="st", name="zptail")
                zpl = zp[:, 0:512]
            isl = slice(ICB + ch * 512, ICB + (ch + 1) * 512)
            for k in range(DH // P):
                nc.tensor.matmul(
                    zpl,
                    wo[k][:, ft * P:(ft + 1) * P],
                    ot[k][:, isl],
                    start=(k == 0),
                    stop=(k == DH // P - 1),
                )
            zsb = zs_pool.tile([P, 512], f32, tag="zt", name="zsb")
            if use_act:
                nc.scalar.activation(zsb[:], zpl, AF.Identity,
                                     bias=zb_sb[:, ft:ft + 1])
            else:
                nc.vector.tensor_scalar_add(zsb[:], zpl, zb_sb[:, ft:ft + 1])
            nc.sync.dma_start(zt_d[ft * P:(ft + 1) * P, isl], zsb[:])

        # 3-wide interleave (two ps_s bufs + pz): transposes first, then z
        # chunks rotating through three psum slots with ACT/DVE-split bias
        tail_transp(0)
        tail_z(0, 0, True)
        tail_transp(1)
        tail_z(1, 0, False)
        tail_z(0, 1, True)
        tail_z(2, 0, False)
        tail_z(1, 1, True)
        tail_z(3, 0, False)
        tail_z(2, 1, True)
        # last chunk split in two halves: biases run on ACT and DVE in
        # parallel and the first DMA overlaps the second bias
        zp = ps_s.tile([P, ICB], f32, tag="st", name="zptail")
        for hh in range(2):
            c0 = hh * 256
            isl = slice(ICB + 512 + c0, ICB + 512 + c0 + 256)
            for k in range(DH // P):
                nc.tensor.matmul(
                    zp[:, c0:c0 + 256],
                    wo[k][:, 3 * P:4 * P],
                    ot[k][:, isl],
                    start=(k == 0),
                    stop=(k == DH // P - 1),
                )
            zsb = zs_pool.tile([P, 512], f32, tag="zt", name="zsb")
            if hh == 0:
                nc.scalar.activation(zsb[:, 0:256], zp[:, c0:c0 + 256],
                                     AF.Identity, bias=zb_sb[:, 3:4])
            else:
                nc.vector.tensor_scalar_add(zsb[:, 0:256], zp[:, c0:c0 + 256],
                                            zb_sb[:, 3:4])
            nc.sync.dma_start(zt_d[3 * P:4 * P, isl], zsb[:, 0:256])

    nc.compile()
    return nc


def _get_nc():
    if "nc" not in _cache:
        _cache["nc"] = build()
    return _cache["nc"]


def make_in_maps(x, Wq, bq, Wk, bk, Wv, bv, Wo, bo):
    """Host-side sharding: per-core input dict for core c = 2*b + hg."""
    bfnp = _bf_np()
    in_maps = []
    for c in range(8):
        b, hg = divmod(c, 2)
        cs = slice(hg * DH, (hg + 1) * DH)
        wo_s = np.ascontiguousarray(Wo[cs, :])
        zb = np.asarray(bv[cs] @ wo_s, dtype=np.float32)
        in_maps.append({
            "xtb": np.ascontiguousarray(np.asarray(x[b]).T.astype(bfnp)),
            "wkqv": np.ascontiguousarray(
                np.concatenate([Wk[:, cs], Wq[:, cs], Wv[:, cs]], axis=1)
                .astype(bfnp)),
            "wob": np.ascontiguousarray(wo_s.astype(bfnp)),
            "bias8": np.ascontiguousarray(np.concatenate([
                np.asarray(bk[cs]).reshape(NPAIR, P).T,
                np.asarray(bq[cs]).reshape(NPAIR, P).T,
                zb.reshape(F // P, P).T,
            ], axis=1).astype(np.float32)),
            "ident": np.eye(P, dtype=bfnp),
        })
    return in_maps


def kernel(x, Wq, bq, Wk, bk, Wv, bv, Wo, bo):
    from concourse.bass_utils import run_bass_kernel_spmd

    x = np.asarray(x, dtype=np.float32)
    args = [np.asarray(a, dtype=np.float32) for a in (Wq, bq, Wk, bk, Wv, bv, Wo, bo)]
    nc = _get_nc()
    in_maps = make_in_maps(x, *args)
    res = run_bass_kernel_spmd(nc, in_maps, list(range(8)))
    bo = args[-1]
    out = np.empty((B, N, F), dtype=np.float32)
    for b in range(B):
        zt0 = res.results[2 * b]["zt"]
        zt1 = res.results[2 * b + 1]["zt"]
        out[b] = (zt0 + zt1).T + bo
    return out


# revision 58
# speedup vs baseline: 1.0036x; 1.0036x over previous
"""Multi-head self-attention TRN2 Bass kernel.

Problem: x[4,2048,512], 8 heads of d=64, scale 1/sqrt(512) (full feature dim).

Sharding: 8 cores = (batch b in 0..3) x (head-group hg in 0..1). Each core
handles one batch element and 4 heads (256 of the 512 features), computing a
partial output projection z_partial = attn_heads @ Wo[hg rows].  The host
sums the two partials per batch and adds bo.

Per-core dataflow (ACT exp is the bottleneck; PE work minimized):
  prologue: xt [512,2048] (host pre-transposed, bf16) and packed bf16
            W_{k,q,v} streamed in; QT/KT = W^T x^T per head-pair [128, n]
            (+bias per partition, accumulated f32);
            V per j-tile [128, 260] bf16 with a ones column per head.
  rounds (icb 0..1 i-chunks of 1024) x (head pair p 0..1), jt 0..15:
    S^T_h [128j, 1024i] = K_h Q_h^T  (two heads row-packed via tile_position)
    P^T_h = exp(S^T_h / sqrt(512))   (ScalarE -> bf16 SBUF)
    O[i_sub, 65] += P^T_slice.T V_ext  (lhsT = P^T 128x128 slice, rhs = V_ext
       [128, 65]; col 64 accumulates the softmax row-sum -> per-partition!)
  normalize: rcp = 1/O[:,64] (DVE [128,1]); O_norm = O[:, :64] * rcp
    (tensor_scalar, per-partition broadcast) -> staged [128 i, 128 d] bf16.
  O^T via identity-matmul transpose (PE) + DVE copy -> ot[kt][128 d, n] bf16;
  z^T[f,i] = Wo^T O^T (bf16) + bias, DMA out per 512-chunk.

Scheduling: ACT must never stall. S(jt+1) is emitted before PV(jt); all other
work (projections, transposes of the previous round, z chunks) lives in a
background FIFO popped once per jt AFTER S(jt+1,h1), so a stalled item can
only delay work that has >1 jt of slack. Background items chain through the
single spare PSUM bank (pz); items are ordered so each completes before its
consumer's deadline.

Output zt [512, 2048] = z^T; host transposes back, sums partials, adds bo.
"""

import sys
import os

sys.path.insert(0, "/opt/trn_rl_repo")

import numpy as np

B, N, F = 4, 2048, 512
H, D = 8, 64
P = 128
DH = 256   # features per core (4 heads)
NPAIR = 2  # head pairs per core
KT = F // P          # 4 k-tiles over input features
ICB = 1024           # i-chunk per round
NICB = N // ICB      # 2
NT = N // P          # 16 j-tiles
NSUB = ICB // P      # 8 i-subtiles per chunk
SCALE = 1.0 / float(np.float32(F) ** 0.5)

_cache = {}


def _bf_np():
    import ml_dtypes

    return np.dtype(ml_dtypes.bfloat16)


def build():
    """Build + bass-compile the per-core program."""
    import concourse.tile as tile
    from concourse import bacc, mybir
    from contextlib import ExitStack

    f32 = mybir.dt.float32
    f32r = mybir.dt.float32r
    bf = mybir.dt.bfloat16
    AF = mybir.ActivationFunctionType

    n = N
    nc = bacc.Bacc("TRN2", target_bir_lowering=False, debug=False)

    xt_d = nc.dram_tensor("xtb", [F, n], bf, kind="ExternalInput").ap()
    wkqv_d = nc.dram_tensor("wkqv", [F, 3 * DH], bf, kind="ExternalInput").ap()
    wo_d = nc.dram_tensor("wob", [DH, F], bf, kind="ExternalInput").ap()
    # packed biases: cols 0-1 bk, 2-3 bq, 4-7 bv@Wo (one DMA, not three)
    bias_d = nc.dram_tensor("bias8", [P, 8], f32, kind="ExternalInput").ap()
    id_d = nc.dram_tensor("ident", [P, P], bf, kind="ExternalInput").ap()
    zt_d = nc.dram_tensor("zt", [F, n], f32, kind="ExternalOutput").ap()

    with tile.TileContext(nc) as tc, ExitStack() as ctx:
        const = ctx.enter_context(tc.tile_pool(name="const", bufs=1))
        pt_pool = ctx.enter_context(tc.tile_pool(name="pt", bufs=6))
        rc_pool = ctx.enter_context(tc.tile_pool(name="rc", bufs=8))
        on_pool = ctx.enter_context(tc.tile_pool(name="on", bufs=16))
        zs_pool = ctx.enter_context(tc.tile_pool(name="zs", bufs=4))
        ps_s = ctx.enter_context(tc.tile_pool(name="ps_s", bufs=2, space="PSUM"))
        po_pool = ctx.enter_context(tc.tile_pool(name="po", bufs=3, space="PSUM"))
        pz_pool = ctx.enter_context(tc.tile_pool(name="pz", bufs=1, space="PSUM"))

        # ---- DMA loads: bf16 x and packed bf16 W_{k,q,v} minimize both the
        # transfer bytes and the HWDGE-serialized DMA count on the critical
        # path to the first exp ----------------------------------------------
        xt = [const.tile([P, n], bf, tag=f"xt{k}", name=f"xt{k}") for k in range(KT)]
        wkqv = [const.tile([P, 3 * DH], bf, tag=f"wkqv{k}", name=f"wkqv{k}")
                for k in range(KT)]
        wk = [wkqv[k][:, 0:DH] for k in range(KT)]
        wq = [wkqv[k][:, DH:2 * DH] for k in range(KT)]
        wv = [wkqv[k][:, 2 * DH:3 * DH] for k in range(KT)]
        bias_sb = const.tile([P, 8], f32, tag="bias8", name="bias_sb")
        bk_sb = bias_sb[:, 0:NPAIR]
        bq_sb = bias_sb[:, NPAIR:2 * NPAIR]
        zb_sb = bias_sb[:, 4:8]
        # warm tiles first: a dummy matmul starts the PE p-state ramp at t~1us
        # so the prologue projections run at full clock, and the exp warms the
        # ScalarE table while DMAs stream in
        warm = const.tile([1, 1], f32, tag="warm", name="warm")
        nc.vector.memset(warm[:], 0.0)
        warmmm = pz_pool.tile([P, 512], f32, tag="pz", name="warmmm")
        nc.tensor.matmul(warmmm[0:1, 0:1], warm[:], warm[:],
                         start=True, stop=True)
        nc.scalar.activation(warm[:], warm[:], AF.Exp)

        nc.sync.dma_start(xt[0][:, 0:ICB], xt_d[0:P, 0:ICB])
        nc.sync.dma_start(wkqv[0][:], wkqv_d[0:P, :])
        nc.sync.dma_start(bias_sb[:], bias_d[:])
        for k in range(1, KT):
            nc.sync.dma_start(xt[k][:, 0:ICB], xt_d[k * P:(k + 1) * P, 0:ICB])
            nc.sync.dma_start(wkqv[k][:], wkqv_d[k * P:(k + 1) * P, :])
        for k in range(KT):
            nc.sync.dma_start(xt[k][:, 1024:n], xt_d[k * P:(k + 1) * P, 1024:n])
        wo = [const.tile([P, F], bf, tag=f"wo{k}", name=f"wo{k}") for k in range(DH // P)]
        for k in range(DH // P):
            nc.sync.dma_start(wo[k][:], wo_d[k * P:(k + 1) * P, :])
        ident = const.tile([P, P], bf, tag="ident", name="ident")
        nc.sync.dma_start(ident[:], id_d[:])

        # persistent activations
        qt = [const.tile([P, n], f32r, tag=f"qt{p}", name=f"qt{p}") for p in range(NPAIR)]
        kt_sb = [const.tile([P, n], f32r, tag=f"kt{p}", name=f"ktsb{p}") for p in range(NPAIR)]
        # V per j-tile: [128, 260] bf16, head hl at cols [65*hl, 65*hl+64),
        # ones at col 65*hl+64 (accumulates softmax row-sums in PV).
        v_sb = [const.tile([P, 4 * (D + 1)], bf, tag=f"v{j}", name=f"v{j}")
                for j in range(NT)]
        for j in range(NT):
            nc.gpsimd.memset(v_sb[j][:], 1.0)
        # O^T staging for the z projection: [128 d, n] bf16 per k-tile (pair)
        ot = [const.tile([P, n], bf, tag=f"ot{p}", name=f"ot{p}") for p in range(NPAIR)]

        def v4(ap):
            return ap.rearrange("p (h c) -> p h c", h=4)

        # ---- projections -----------------------------------------------------
        def proj_qk_half(p, w_t, b_sb, dst, ib, half, pool, act_bias=False):
            """One 512-wide half of a Q/K projection chunk."""
            ps = pool.tile([P, 512], f32, tag="pz", name="pjh") if pool is pz_pool \
                else pool.tile([P, ICB], f32, tag="st", name="pjs")
            psl = ps[:, 0:512]
            isl = slice(ib * ICB + half * 512, ib * ICB + (half + 1) * 512)
            for k in range(KT):
                nc.tensor.matmul(
                    psl,
                    w_t[k][:, p * P:(p + 1) * P],
                    xt[k][:, isl],
                    start=(k == 0),
                    stop=(k == KT - 1),
                )
            if act_bias:
                # prologue: ACT is idle and this sits on the first-exp path
                nc.scalar.activation(dst[p][:, isl], psl, AF.Identity,
                                     bias=b_sb[:, p:p + 1])
            else:
                nc.vector.tensor_scalar_add(dst[p][:, isl], psl, b_sb[:, p:p + 1])

        def proj_v_pair(j0, pool):
            """V for j-tiles j0, j0+1 sharing one pz bank (two 256-col slices)."""
            ps = pool.tile([P, 512], f32, tag="pz", name="pvh") if pool is pz_pool \
                else pool.tile([P, ICB], f32, tag="st", name="pvs")
            for m, j in enumerate((j0, j0 + 1)):
                psl = ps[:, m * DH:(m + 1) * DH]
                for k in range(KT):
                    # one start=True per pz bank; the second slice's first
                    # matmul writes pending-zero bytes (zeroed on write)
                    nc.tensor.matmul(
                        psl,
                        xt[k][:, j * P:(j + 1) * P],
                        wv[k][:],
                        start=(k == 0 and m == 0),
                        stop=(k == KT - 1 and m == 1),
                        skip_group_check=True,
                    )
            for m, j in enumerate((j0, j0 + 1)):
                psl = ps[:, m * DH:(m + 1) * DH]
                nc.vector.tensor_copy(v4(v_sb[j][:])[:, :, 0:D], v4(psl))

        def z_chunk(icb, ft, ch, pool):
            """z^T[ft*128:(ft+1)*128, 512-chunk ch of icb]."""
            if pool is pz_pool:
                zp = pool.tile([P, 512], f32, tag="pz", name="zp")
                zpl = zp[:, 0:512]
            else:
                zp = pool.tile([P, ICB], f32, tag="st", name="zps")
                zpl = zp[:, 0:512]
            isl = slice(icb * ICB + ch * 512, icb * ICB + (ch + 1) * 512)
            for k in range(DH // P):
                nc.tensor.matmul(
                    zpl,
                    wo[k][:, ft * P:(ft + 1) * P],
                    ot[k][:, isl],
                    start=(k == 0),
                    stop=(k == DH // P - 1),
                )
            zsb = zs_pool.tile([P, 512], f32, tag="zt", name="zsb")
            nc.vector.tensor_scalar_add(zsb[:], zpl, zb_sb[:, ft:ft + 1])
            nc.sync.dma_start(zt_d[ft * P:(ft + 1) * P, isl], zsb[:])

        def transp_group(p, icb, onst, grp):
            """Transpose 4 O_norm [128,64] blocks -> ot via one pz bank."""
            tp = pz_pool.tile([P, 512], f32, tag="pz", name="tp")
            for i, (h, sub) in enumerate(grp):
                nc.tensor.matmul(
                    tp[0:D, i * P:(i + 1) * P],
                    onst[sub][:, h * D:(h + 1) * D],
                    ident[:],
                    start=(i == 0),
                    stop=(i == 3),
                    skip_group_check=True,
                )
            for i, (h, sub) in enumerate(grp):
                dst = ot[p][h * D:(h + 1) * D,
                            icb * ICB + sub * P: icb * ICB + (sub + 1) * P]
                nc.vector.tensor_copy(dst, tp[0:D, i * P:(i + 1) * P])

        # ---- attention rounds ------------------------------------------------
        def ob_slice(ob, h, sub):
            """PSUM accumulator slice [128, 65] for (head h, i-subtile sub)."""
            if sub < 7:
                t = ob[h]
                c0 = sub * (D + 1)
            else:
                t = ob[2]
                c0 = h * (D + 1)
            return t[:, c0:c0 + D + 1]

        def emit_S(p, icb, jt, h):
            sps = ps_s.tile([P, ICB], f32, tag="st", name="sps")
            hp = slice(D * h, D * (h + 1))
            for i5 in range(2):
                isl = slice(icb * ICB + i5 * 512, icb * ICB + (i5 + 1) * 512)
                nc.tensor.matmul(
                    sps[:, i5 * 512:(i5 + 1) * 512],
                    kt_sb[p][hp, jt * P:(jt + 1) * P],
                    qt[p][hp, isl],
                    start=True,
                    stop=True,
                    tile_position=(D * h, 0),
                )
            return sps

        # prologue: K/Q first chunks + the first S tile + the FIRST EXPS,
        # emitted as early as possible in the stream (the exps must precede
        # all other PE work in emission order or they inherit its position).
        # K-h1 goes through pz so the ps_s slots stay clear for the S tiles.
        proj_qk_half(0, wq, bq_sb, qt, 0, 0, ps_s, act_bias=True)  # slot A
        proj_qk_half(0, wk, bk_sb, kt_sb, 0, 0, ps_s)   # slot B
        sps00 = ps_s.tile([P, ICB], f32, tag="st", name="sps00")  # slot A
        nc.tensor.matmul(
            sps00[:, 0:512], kt_sb[0][0:D, 0:P], qt[0][0:D, 0:512],
            start=True, stop=True, tile_position=(0, 0),
        )
        ptile00 = pt_pool.tile([P, ICB], bf, tag="pt", name="pt00")
        nc.scalar.activation(ptile00[:, 0:512], sps00[:, 0:512],
                             AF.Exp, scale=SCALE)
        proj_qk_half(0, wq, bq_sb, qt, 0, 1, ps_s, act_bias=True)  # slot B
        nc.tensor.matmul(
            sps00[:, 512:1024], kt_sb[0][0:D, 0:P], qt[0][0:D, 512:1024],
            start=True, stop=True, tile_position=(0, 0),
        )
        proj_qk_half(0, wk, bk_sb, kt_sb, 0, 1, pz_pool)
        nc.scalar.activation(ptile00[:, 512:1024], sps00[:, 512:1024],
                             AF.Exp, scale=SCALE)
        sps01 = emit_S(0, 0, 0, 1)
        ptile01 = pt_pool.tile([P, ICB], bf, tag="pt", name="pt01")
        nc.scalar.activation(ptile01[:], sps01[:], AF.Exp, scale=SCALE)

        rounds = [(0, 0), (0, 1), (1, 0), (1, 1)]
        bg = []          # background FIFO: fns emitting pz-chained work
        onst_by_round = {}

        for r, (icb, p) in enumerate(rounds):
            # load this round's background work (deadline-ordered)
            if r == 0:
                bg += [
                    lambda: proj_v_pair(0, pz_pool),
                    lambda: proj_v_pair(2, pz_pool),
                    lambda: proj_v_pair(4, pz_pool),
                    lambda: proj_v_pair(6, pz_pool),
                    lambda: proj_v_pair(8, pz_pool),
                    lambda: proj_qk_half(0, wk, bk_sb, kt_sb, 1, 0, pz_pool),
                    lambda: proj_qk_half(0, wk, bk_sb, kt_sb, 1, 1, pz_pool),
                    lambda: proj_v_pair(10, pz_pool),
                    lambda: proj_v_pair(12, pz_pool),
                    lambda: proj_qk_half(1, wk, bk_sb, kt_sb, 0, 0, pz_pool),
                    lambda: proj_qk_half(1, wq, bq_sb, qt, 0, 0, pz_pool),
                    lambda: proj_v_pair(14, pz_pool),
                    lambda: proj_qk_half(1, wk, bk_sb, kt_sb, 0, 1, pz_pool),
                    lambda: proj_qk_half(1, wq, bq_sb, qt, 0, 1, pz_pool),
                ]
            elif r == 1:
                po_, oo_ = onst_by_round[0]
                items = [(h, s) for h in range(2) for s in range(NSUB)]
                bg += [
                    lambda: proj_qk_half(1, wk, bk_sb, kt_sb, 1, 0, pz_pool),
                    lambda: proj_qk_half(1, wk, bk_sb, kt_sb, 1, 1, pz_pool),
                ]
                bg += [lambda g=g, po2=po_, oo2=oo_: transp_group(
                        po2[0], po2[1], oo2, items[g * 4:(g + 1) * 4])
                       for g in range(4)]
                bg += [
                    lambda: proj_qk_half(0, wq, bq_sb, qt, 1, 0, pz_pool),
                    lambda: proj_qk_half(0, wq, bq_sb, qt, 1, 1, pz_pool),
                    lambda: proj_qk_half(1, wq, bq_sb, qt, 1, 0, pz_pool),
                    lambda: proj_qk_half(1, wq, bq_sb, qt, 1, 1, pz_pool),
                ]
            elif r == 2:
                po_, oo_ = onst_by_round[1]
                items = [(h, s) for h in range(2) for s in range(NSUB)]
                bg += [lambda g=g, po2=po_, oo2=oo_: transp_group(
                        po2[0], po2[1], oo2, items[g * 4:(g + 1) * 4])
                       for g in range(4)]
                bg += [lambda ft=ft, ch=ch: z_chunk(0, ft, ch, pz_pool)
                       for ft in range(F // P) for ch in range(2)]
            elif r == 3:
                po_, oo_ = onst_by_round[2]
                items = [(h, s) for h in range(2) for s in range(NSUB)]
                bg += [lambda g=g, po2=po_, oo2=oo_: transp_group(
                        po2[0], po2[1], oo2, items[g * 4:(g + 1) * 4])
                       for g in range(4)]

            ob = [po_pool.tile([P, 512], f32, tag="po", name=f"ob{i}")
                  for i in range(3)]
            if r == 0:
                sps_tiles = {0: sps00, 1: sps01}
            else:
                sps_tiles = {0: emit_S(p, icb, 0, 0), 1: emit_S(p, icb, 0, 1)}
            for jt in range(NT):
                nxt = {}
                if jt + 1 < NT:
                    nxt[0] = emit_S(p, icb, jt + 1, 0)
                    # slot 1: only pop when backlog exceeds remaining jts
                    if bg and len(bg) > (NT - jt):
                        bg.pop(0)()
                    nxt[1] = emit_S(p, icb, jt + 1, 1)
                if bg:
                    bg.pop(0)()
                for h in range(2):
                    hl = 2 * p + h
                    if r == 0 and jt == 0:
                        # exps pre-emitted in the prologue
                        ptile = (ptile00, ptile01)[h]
                    else:
                        ptile = pt_pool.tile([P, ICB], bf, tag="pt", name="pt")
                        nc.scalar.activation(ptile[:], sps_tiles[h][:], AF.Exp, scale=SCALE)
                    for sub in range(NSUB):
                        # start=True zeroes the whole 2KB psum bank row, so
                        # only the first matmul into each ob bank may set it
                        first_in_bank = (sub == 0) or (sub == 7 and h == 0)
                        nc.tensor.matmul(
                            ob_slice(ob, h, sub),
                            ptile[:, sub * P:(sub + 1) * P],
                            v_sb[jt][:, hl * (D + 1):(hl + 1) * (D + 1)],
                            start=(jt == 0 and first_in_bank),
                            stop=(jt == NT - 1),
                            skip_group_check=True,
                        )
                sps_tiles = nxt

            # ---- normalize (frees the ob banks for the next round) ----------
            onst = [on_pool.tile([P, P], bf, tag="on", name=f"on{s}")
                    for s in range(NSUB)]
            for h in range(2):
                # batched reciprocals: subs 0-6 rowsums (col 64 of each
                # 65-col slice in bank ob[h]) in one strided op, sub 7 single
                rc8 = rc_pool.tile([P, 8], f32, tag="rc8", name="rc8")
                in7 = ob[h][:, 0:7 * (D + 1)].rearrange(
                    "p (s c) -> p s c", c=D + 1)[:, :, D:D + 1]
                out7 = rc8[:, 0:7].rearrange("p (s c) -> p s c", c=1)
                nc.vector.reciprocal(out7, in7)
                nc.vector.reciprocal(
                    rc8[:, 7:8], ob[2][:, h * (D + 1) + D: h * (D + 1) + D + 1])
                for sub in range(NSUB):
                    sl = ob_slice(ob, h, sub)
                    rc = rc8[:, sub:sub + 1]
                    dst = onst[sub][:, h * D:(h + 1) * D]
                    if r == 3 and sub % 2 == h:
                        # tail: ACT is idle -> per-partition scale via Copy
                        nc.scalar.activation(dst, sl[:, 0:D], AF.Copy, scale=rc)
                    else:
                        nc.vector.tensor_scalar_mul(dst, sl[:, 0:D], rc)
            onst_by_round[r] = ((p, icb), onst)

        # ---- tail: last round's transposes via the freed wide ps_s tiles,
        # then icb=1 z chunks rotating through pz + both ps_s bufs ------------
        _, onst3 = onst_by_round[3]

        def tail_transp(half):
            """Transpose subs [4*half, 4*half+4) x both heads -> ot."""
            tp = ps_s.tile([P, ICB], f32, tag="st", name="tptail")
            grp8 = [(h, s) for h in range(2)
                    for s in range(half * 4, half * 4 + 4)]
            for i, (h, sub) in enumerate(grp8):
                nc.tensor.matmul(
                    tp[0:D, i * P:(i + 1) * P],
                    onst3[sub][:, h * D:(h + 1) * D],
                    ident[:],
                    start=(i % 4 == 0),
                    stop=(i % 4 == 3),
                    skip_group_check=True,
                )
            for i, (h, sub) in enumerate(grp8):
                dst = ot[1][h * D:(h + 1) * D,
                            ICB + sub * P: ICB + (sub + 1) * P]
                if i % 2 == 0:
                    nc.scalar.activation(dst, tp[0:D, i * P:(i + 1) * P], AF.Copy)
                else:
                    nc.vector.tensor_copy(dst, tp[0:D, i * P:(i + 1) * P])

        def tail_z(ft, ch, use_act, pool=None):
            """One [128,512] z chunk of icb=1 with ACT- or DVE-side bias."""
            if pool is pz_pool:
                zp = pz_pool.tile([P, 512], f32, tag="pz", name="zptail")
                zpl = zp[:, 0:512]
            else:
                zp = ps_s.tile([P, ICB], f32, tag="st", name="zptail")
                zpl = zp[:, 0:512]
            isl = slice(ICB + ch * 512, ICB + (ch + 1) * 512)
            for k in range(DH // P):
                nc.tensor.matmul(
                    zpl,
                    wo[k][:, ft * P:(ft + 1) * P],
                    ot[k][:, isl],
                    start=(k == 0),
                    stop=(k == DH // P - 1),
                )
            zsb = zs_pool.tile([P, 512], f32, tag="zt", name="zsb")
            if use_act:
                nc.scalar.activation(zsb[:], zpl, AF.Identity,
                                     bias=zb_sb[:, ft:ft + 1])
            else:
                nc.vector.tensor_scalar_add(zsb[:], zpl, zb_sb[:, ft:ft + 1])
            nc.sync.dma_start(zt_d[ft * P:(ft + 1) * P, isl], zsb[:])

        # 3-wide interleave (two ps_s bufs + pz): transposes first, then z
        # chunks rotating through three psum slots with ACT/DVE-split bias
        tail_transp(0)
        tail_z(0, 0, True)
        tail_transp(1)
        tail_z(1, 0, False)
        tail_z(0, 1, True)
        tail_z(2, 0, False)
        tail_z(1, 1, True)
        tail_z(3, 0, False)
        tail_z(2, 1, True)
        tail_z(3, 1, False)

    nc.compile()
    return nc


def _get_nc():
    if "nc" not in _cache:
        _cache["nc"] = build()
    return _cache["nc"]


def make_in_maps(x, Wq, bq, Wk, bk, Wv, bv, Wo, bo):
    """Host-side sharding: per-core input dict for core c = 2*b + hg."""
    bfnp = _bf_np()
    in_maps = []
    for c in range(8):
        b, hg = divmod(c, 2)
        cs = slice(hg * DH, (hg + 1) * DH)
        wo_s = np.ascontiguousarray(Wo[cs, :])
        zb = np.asarray(bv[cs] @ wo_s, dtype=np.float32)
        in_maps.append({
            "xtb": np.ascontiguousarray(np.asarray(x[b]).T.astype(bfnp)),
            "wkqv": np.ascontiguousarray(
                np.concatenate([Wk[:, cs], Wq[:, cs], Wv[:, cs]], axis=1)
                .astype(bfnp)),
            "wob": np.ascontiguousarray(wo_s.astype(bfnp)),
            "bias8": np.ascontiguousarray(np.concatenate([
                np.asarray(bk[cs]).reshape(NPAIR, P).T,
                np.asarray(bq[cs]).reshape(NPAIR, P).T,
                zb.reshape(F // P, P).T,
            ], axis=1).astype(np.float32)),
            "ident": np.eye(P, dtype=bfnp),
        })
    return in_maps


def kernel(x, Wq, bq, Wk, bk, Wv, bv, Wo, bo):
    from concourse.bass_utils import run_bass_kernel_spmd

    x = np.asarray(x, dtype=np.float32)
    args = [np.asarray(a, dtype=np.float32) for a in (Wq, bq, Wk, bk, Wv, bv, Wo, bo)]
    nc = _get_nc()
    in_maps = make_in_maps(x, *args)
    res = run_bass_kernel_spmd(nc, in_maps, list(range(8)))
    bo = args[-1]
    out = np.empty((B, N, F), dtype=np.float32)
    for b in range(B):
        zt0 = res.results[2 * b]["zt"]
        zt1 = res.results[2 * b + 1]["zt"]
        out[b] = (zt0 + zt1).T + bo
    return out
